# revision 18
# baseline (speedup 1.0000x reference)
"""Trainium2 Bass kernel for a NonLocalBlock (B=2, C=256, H=W=64).

Math (reference):
    theta/phi/g = 1x1 conv of inp (C -> CH=128), L = CH*H = 8192
    attn = softmax(th @ ph^T) over kv, with th, ph (L, W=64)
    o = attn @ gg -> out = conv1x1(o) + b_out + inp

Sharding: 8 cores = 2 samples x 4 h-blocks (16 h rows each). Each core
computes the attention output rows q=(ch, h) for its h-block, which is
exactly the data the final conv needs for output columns (h, w) of its
block, so there is no cross-core reduction.

The per-core x is column-permuted so the core's own 16 h rows come
first: the kernel is then identical on every core (SPMD) — softmax over
kv is permutation-invariant as long as phT and V use the same h order,
and both are derived from the same permuted x.

Per-core layouts (kv order = (h', ch') so V tiles come straight from the
g conv output; q order = (h, ch) so attention output transposes directly
into (ch, (h, w)) for the out conv):
    Qt  (64=w,  2048=q)    phT (64=w, 8192=kv)    vaug (128=ch', 64*65)
    S^T tile t = phT[:, t*128:(t+1)*128].T @ Qt   -> psum (128=kv, q)
    E = exp(S^T)  (no max subtraction: logits are within +-80 in fp32)
    O'^T += E.T @ [V_t | 1]  -> psum (65, q): rows 0..63 = o, row 64 = denom

dtypes: Q/K/V fp16 (10-bit mantissa ~ fp32r accuracy, 1 cyc/row + fast
weight load), E bf16 (needs fp32 exponent range: E spans e^+-70), all
matmul accumulation fp32 in PSUM, logits fp32, out conv fp32r.

Steady state is ACT-bound: exp of a (128,1024) fp32 PSUM tile takes
~1113ns and runs back-to-back for 128 iterations (~148us). The wins over
that floor are in the prologue (x DMA split into critical 512-col chunks
across the three DMA queues; conv bias folded into the matmul via a
K=1 ones-row so the psum->sbuf move is a single strided copy writing the
qt and pht blocks of one merged tile; PE warm-up matmuls to start the
HAM clock ramp during the DMA wait) and the drain (otsb copies split
across DVE/ACT as soon as each PV chunk retires, normalize spread over
ACT+DVE, y stores alternating sync/scalar queues).
"""

import numpy as np

B, C, H, W = 2, 256, 64, 64
CH = C // 2          # 128
HS = H // 4          # 16 h rows per core
LQ = CH * HS         # 2048 q rows per core
NKV = 64             # kv tiles of 128 (kv = (h', ch'))
QP = 1024            # q per attention pass (PSUM budget)
PHB = 2048           # pht column base inside the merged qph tile

_cached = {}


def _build_nc():
    import concourse.mybir as mybir
    import concourse.tile as tile
    from concourse import bacc

    f32 = mybir.dt.float32
    f32r = mybir.dt.float32r
    f16 = mybir.dt.float16
    bf16 = mybir.dt.bfloat16
    AF = mybir.ActivationFunctionType
    ALU = mybir.AluOpType

    nc = bacc.Bacc("TRN2", target_bir_lowering=False, debug=False, num_devices=8)

    x0 = nc.dram_tensor("x0", [128, 4096], f16, kind="ExternalInput")
    x1 = nc.dram_tensor("x1", [128, 4096], f16, kind="ExternalInput")
    xs0 = nc.dram_tensor("xs0", [128, 1024], f32, kind="ExternalInput")
    xs1 = nc.dram_tensor("xs1", [128, 1024], f32, kind="ExternalInput")
    wc = nc.dram_tensor("wc", [128, 2, 256], f16, kind="ExternalInput")
    wg = nc.dram_tensor("wg", [128, 2, 128], f16, kind="ExternalInput")
    wo = nc.dram_tensor("wo", [128, 2, 128], f32r, kind="ExternalInput")
    bias2 = nc.dram_tensor("bias2", [64, 256], f32, kind="ExternalInput")
    bg = nc.dram_tensor("bg", [128, 1], f32, kind="ExternalInput")
    bo = nc.dram_tensor("bo", [128, 2], f32, kind="ExternalInput")
    ident = nc.dram_tensor("ident", [128, 128], f32, kind="ExternalInput")
    y = nc.dram_tensor("y", [2, 128, 1024], f32, kind="ExternalOutput")

    with tile.TileContext(nc) as tc:
        with (
            tc.tile_pool(name="const", bufs=1) as cp,
            tc.tile_pool(name="big", bufs=1) as bp,
            tc.tile_pool(name="work", bufs=3) as wkp,
            tc.tile_pool(name="psum", bufs=1, space="PSUM") as pp,
        ):
            x0t = bp.tile([128, 4096], f16, tag="x0")
            x1t = bp.tile([128, 4096], f16, tag="x1")
            xs0t = bp.tile([128, 1024], f32, tag="xs0")
            xs1t = bp.tile([128, 1024], f32, tag="xs1")
            wct = cp.tile([128, 2, 256], f16, tag="wc")
            wgt = cp.tile([128, 2, 128], f16, tag="wg")
            wot = cp.tile([128, 2, 128], f32r, tag="wo")
            b2t = cp.tile([64, 256], f32, tag="bias2")
            bgt = cp.tile([128, 1], f32, tag="bg")
            bot = cp.tile([128, 2], f32, tag="bo")
            idt = cp.tile([128, 128], f32, tag="ident")
            warm16 = cp.tile([128, 512], f16, tag="warm16")

            # critical-path loads first; DMA issue to visible-data latency
            # is ~3us and each queue serializes, so the first chunks are
            # tiny and the wc weight halves split across the two HW queues
            nc.sync.dma_start(out=x0t[:, 0:128], in_=x0[:, 0:128])
            nc.scalar.dma_start(out=x1t[:, 0:128], in_=x1[:, 0:128])
            nc.sync.dma_start(out=wct[:, 0, :], in_=wc[:, 0, :])
            nc.scalar.dma_start(out=wct[:, 1, :], in_=wc[:, 1, :])
            nc.gpsimd.dma_start(out=b2t[:], in_=bias2[:])
            nc.gpsimd.dma_start(out=wgt[:], in_=wg[:])
            nc.gpsimd.dma_start(out=bgt[:], in_=bg[:])
            nc.sync.dma_start(out=x0t[:, 128:512], in_=x0[:, 128:512])
            nc.scalar.dma_start(out=x1t[:, 128:512], in_=x1[:, 128:512])
            # rest of the first halves; hp4..15 consume these from ~it 2.
            nc.sync.dma_start(out=x0t[:, 512:2048], in_=x0[:, 512:2048])
            nc.scalar.dma_start(out=x1t[:, 512:2048], in_=x1[:, 512:2048])

            def emit_late_dmas(step):
                # second halves + residuals + out-conv weights ride the
                # sync queue under the attention loop (engines untouched)
                if step == 0:
                    nc.sync.dma_start(out=x0t[:, 2048:4096],
                                      in_=x0[:, 2048:4096])
                elif step == 1:
                    nc.sync.dma_start(out=x1t[:, 2048:4096],
                                      in_=x1[:, 2048:4096])
                elif step == 2:
                    nc.sync.dma_start(out=wot[:], in_=wo[:])
                    nc.sync.dma_start(out=bot[:], in_=bo[:])
                    nc.sync.dma_start(out=idt[:], in_=ident[:])
                elif step == 3:
                    nc.sync.dma_start(out=xs0t[:], in_=xs0[:])
                elif step == 4:
                    nc.sync.dma_start(out=xs1t[:], in_=xs1[:])

            # warm16 feeds the PE warm-up matmuls (HAM unthrottles only
            # after sustained activity — start the ramp while the x DMAs
            # are in flight)
            nc.vector.memset(warm16[:], 0.125)

            # preload the exp table set while DMAs run, so the first
            # attention exp does not stall on ACT_TABLE_LOAD
            warm = wkp.tile([1, 1], f32, tag="warm")
            nc.scalar.activation(warm[:], warm16[0:1, 0:1], AF.Exp)

            for dmy in range(5):
                dps = pp.tile([128, 512], f32, tag="ot", name=f"dmy{dmy}")
                nc.tensor.matmul(dps[:], lhsT=warm16[:, 0:128],
                                 rhs=warm16[:], start=True, stop=True)

            # [w; w] duplicated along partitions so the S matmul runs at
            # K=128: the HAM activity monitor never un-throttles the PE
            # clock (stays 1.2 GHz) for K=64 matmuls, measured directly.
            # The phi half of wc/bc is pre-scaled by 0.5 on the host so
            # the duplicated contraction sums to the original dot product.
            # qt and pht are views of ONE tile so a single strided copy
            # can write a 128-col block of each (theta cols 0:128 of ps,
            # phi cols 128:256, at constant distance PHB in qph).
            qph = bp.tile([128, PHB + 8192], f16, tag="qph")
            qt = qph[:, 0:PHB]                             # (w2, q)
            pht = qph[:, PHB:PHB + 8192]                   # (w2, kv)
            vaug = bp.tile([128, NKV * 65], bf16, tag="vaug")
            osb = bp.tile([128, 1024], f32r, tag="osb")    # o (ch, (h, w))
            otsb = bp.tile([65, LQ], f32, tag="otsb")      # O'^T staged in SBUF
            ysb0 = bp.tile([128, 1024], f32, tag="ysb0")
            ysb1 = bp.tile([128, 1024], f32, tag="ysb1")
            xsb0 = bp.tile([128, 1024], f32, tag="xsb0")
            xsb1 = bp.tile([128, 1024], f32, tag="xsb1")
            vaug3 = vaug.rearrange("p (t j) -> p t j", j=65)
            nc.vector.memset(vaug3[:, :, 64:65], 1.0)
            # 128-col block view: block h holds qt cols, block 16+h the
            # matching pht cols -> strided slice h::16 covers both
            qphb = qph.rearrange("p (a c) -> p a c", c=128)

            # ---- producer emitters (interleaved into the attention loop so
            # the PE/DVE streams overlap attention instead of preceding it) --

            b2v = b2t.rearrange("p (a c) -> p a c", c=128)

            def emit_hp_pro(hp):
                # prologue h-pairs: 2 matmuls, then per half one DVE
                # add that applies [bth|bph/2] and writes the qt and pht
                # 128-col blocks of the merged tile in a single strided op
                ps = pp.tile([128, 256], f32, tag="conv", bufs=2,
                             name=f"c{hp}")
                nc.tensor.matmul(ps[:],
                                 lhsT=x0t[:, hp * 128:(hp + 1) * 128],
                                 rhs=wct[:, 0, :], start=True, stop=False)
                nc.tensor.matmul(ps[:],
                                 lhsT=x1t[:, hp * 128:(hp + 1) * 128],
                                 rhs=wct[:, 1, :], start=False, stop=True)
                ps3 = ps.rearrange("p (a c) -> p a c", c=128)
                for hh in range(2):
                    h = 2 * hp + hh
                    nc.vector.tensor_tensor(
                        out=qphb[0:64, h:h + 17:16, :],
                        in0=ps3[hh * 64:(hh + 1) * 64, :, :],
                        in1=b2v[:], op=ALU.add)
                if hp == 3:
                    # single batched dup of the first q block + first kv
                    # tiles; S(0..3) run K=64 so they don't wait on this
                    nc.gpsimd.dma_start(out=qph[64:128, 0:1024],
                                        in_=qph[0:64, 0:1024])
                    nc.gpsimd.dma_start(out=qph[64:128, PHB:PHB + 1024],
                                        in_=qph[0:64, PHB:PHB + 1024])

            def emit_hp(hp):
                # steady-state h-pairs: classic path (DVE bias adds; DVE
                # has slack under the loop, PE and ACT do not)
                n1 = 256 if hp < 8 else 128
                w0 = 0 if hp < 8 else 128
                ps = pp.tile([128, 256], f32, tag="conv", bufs=2,
                             name=f"c{hp}")
                nc.tensor.matmul(ps[:, 0:n1],
                                 lhsT=x0t[:, hp * 128:(hp + 1) * 128],
                                 rhs=wct[:, 0, w0:w0 + n1],
                                 start=True, stop=False)
                nc.tensor.matmul(ps[:, 0:n1],
                                 lhsT=x1t[:, hp * 128:(hp + 1) * 128],
                                 rhs=wct[:, 1, w0:w0 + n1],
                                 start=False, stop=True)
                pcol = 128 if hp < 8 else 0
                for hh in range(2):
                    h = 2 * hp + hh
                    nc.vector.tensor_tensor(
                        out=pht[0:64, h * 128:(h + 1) * 128],
                        in0=ps[hh * 64:(hh + 1) * 64, pcol:pcol + 128],
                        in1=b2t[:, 128:256], op=ALU.add)
                    if hp < 8:
                        nc.vector.tensor_tensor(
                            out=qt[0:64, h * 128:(h + 1) * 128],
                            in0=ps[hh * 64:(hh + 1) * 64, 0:128],
                            in1=b2t[:, 0:128], op=ALU.add)
                if hp == 7:
                    nc.gpsimd.dma_start(out=qt[64:128, 1024:2048],
                                        in_=qt[0:64, 1024:2048])
                if hp % 2 == 1 and hp > 4:
                    c0 = (hp - 1) * 256
                    nc.gpsimd.dma_start(out=pht[64:128, c0:c0 + 512],
                                        in_=pht[0:64, c0:c0 + 512])

            def emit_g(n, pro=False):
                # g conv chunk -> vaug (values only; ones column pre-memset)
                ps = pp.tile([128, 512], f32, tag="conv", bufs=2,
                             name=f"g{n}")
                nc.tensor.matmul(ps[:], lhsT=wgt[:, 0, :],
                                 rhs=x0t[:, n * 512:(n + 1) * 512],
                                 start=True, stop=False)
                nc.tensor.matmul(ps[:], lhsT=wgt[:, 1, :],
                                 rhs=x1t[:, n * 512:(n + 1) * 512],
                                 start=False, stop=True)
                nc.vector.tensor_scalar(
                    out=vaug3[:, n * 8:(n + 1) * 8, 0:64],
                    in0=ps.rearrange("p (t j) -> p t j", j=64)[:],
                    scalar1=bgt[:, 0:1], scalar2=None, op0=ALU.add)

            def emit_lh(lh, norm_eng):
                # transpose + normalize one 128-q block of O'^T
                trp = pp.tile([128, 65], f32, tag="conv", bufs=2,
                              name=f"tr{lh}")
                nc.tensor.transpose(trp[:],
                                    otsb[:, lh * 128:(lh + 1) * 128],
                                    idt[0:65, 0:65])
                rden = wkp.tile([128, 1], f32, tag="rden", name=f"rd{lh}")
                nc.vector.reciprocal(rden[:], trp[:, 64:65])
                if norm_eng == "act":
                    nc.scalar.activation(osb[:, lh * 64:(lh + 1) * 64],
                                         trp[:, 0:64], AF.Copy,
                                         scale=rden[:])
                else:
                    nc.vector.tensor_scalar(
                        out=osb[:, lh * 64:(lh + 1) * 64],
                        in0=trp[:, 0:64],
                        scalar1=rden[:], scalar2=None, op0=ALU.mult)

            def emit_xsb(m):
                # precombine residual + out-conv bias while attention runs
                xsb = xsb0 if m == 0 else xsb1
                xres = xs0t if m == 0 else xs1t
                nc.vector.tensor_scalar(out=xsb[:], in0=xres[:],
                                        scalar1=bot[:, m:m + 1], scalar2=None,
                                        op0=ALU.add)

            def emit_y(m, c0, c1):
                # out conv for columns [c0:c1] + (bias+residual) + store;
                # stores alternate queues so the tail two run in parallel
                ysb = ysb0 if m == 0 else ysb1
                xsb = xsb0 if m == 0 else xsb1
                yp = pp.tile([128, 512], f32, tag="conv", bufs=2,
                             name=f"yp{m}{c0}")
                nc.tensor.matmul(yp[:, 0:c1 - c0], lhsT=wot[:, m, :],
                                 rhs=osb[:, c0:c1], start=True, stop=True)
                nc.vector.tensor_tensor(
                    out=ysb[:, c0:c1], in0=yp[:, 0:c1 - c0],
                    in1=xsb[:, c0:c1], op=ALU.add)
                eng = nc.sync if m == 0 else nc.scalar
                eng.dma_start(out=y[m, :, c0:c1], in_=ysb[:, c0:c1])

            # ---- attention: software-pipelined over 2 q passes of 1024 ----
            NIT = 2 * NKV
            ets = {}
            otps = {}

            def emit_s(it):
                # the first four kv tiles run K=64 from the single-copy
                # rows (exp scale 2 compensates the pre-halved phi), so
                # the first exps don't wait for the [w;w] dup DMAs
                p, t = it // NKV, it % NKV
                k = 64 if it < 4 else 128
                sp = pp.tile([128, QP], f32, tag="s", bufs=2, name=f"sp{it}")
                for c in range(2):
                    nc.tensor.matmul(
                        sp[:, c * 512:(c + 1) * 512],
                        lhsT=pht[0:k, t * 128:(t + 1) * 128],
                        rhs=qt[0:k, p * QP + c * 512: p * QP + (c + 1) * 512],
                        start=True, stop=True)
                et = wkp.tile([128, QP], bf16, tag="e", bufs=4, name=f"et{it}")
                nc.scalar.activation(et[:], sp[:], AF.Exp,
                                     scale=2.0 if it < 4 else 1.0)
                ets[it] = et

            def emit_pv(it):
                p, t = it // NKV, it % NKV
                if t == 0:
                    otps[p] = pp.tile([65, QP], f32, tag="ot", bufs=1,
                                      name=f"otp{p}")
                otp = otps[p]
                et = ets.pop(it)
                last = t == NKV - 1
                for c in range(2):
                    nc.tensor.matmul(
                        otp[:, c * 512:(c + 1) * 512],
                        lhsT=vaug3[:, t, :],
                        rhs=et[:, c * 512:(c + 1) * 512],
                        start=(t == 0), stop=last,
                        skip_group_check=True)
                    if last:
                        # stage each finished half immediately; the final
                        # pass puts half on ACT (idle once exp is done)
                        eng = nc.scalar if (p == 1 and c == 1) else nc.vector
                        if eng is nc.scalar:
                            nc.scalar.activation(
                                otsb[:, p * QP + c * 512:p * QP + (c + 1) * 512],
                                otp[:, c * 512:(c + 1) * 512], AF.Copy)
                        else:
                            nc.vector.tensor_copy(
                                otsb[:, p * QP + c * 512:p * QP + (c + 1) * 512],
                                otp[:, c * 512:(c + 1) * 512])
                if last and p == 0:
                    for lh in range(8):
                        todo.append(lambda lh=lh: emit_lh(lh, "vec"))
                    todo.append(lambda: emit_xsb(0))
                    todo.append(lambda: emit_xsb(1))
                    todo.append(lambda: emit_y(0, 0, 512))
                    todo.append(lambda: emit_y(1, 0, 512))

            from collections import deque
            todo = deque()
            for hp in range(4):
                emit_hp_pro(hp)
            emit_s(0)
            emit_g(0, pro=True)
            n_hp = 4
            n_g = 1
            for it in range(1, NIT):
                emit_s(it)
                emit_pv(it - 1)
                if it in (1, 3, 5, 7, 9):
                    emit_late_dmas(it // 2)
                if it % 2 == 0 and n_hp < 32:
                    emit_hp(n_hp)
                    n_hp += 1
                if it % 6 == 5 and n_g < 8:
                    emit_g(n_g)
                    n_g += 1
                if todo and it % 2 == 1:
                    todo.popleft()()
            emit_pv(NIT - 1)
            while todo:
                todo.popleft()()
            for lh in range(8, 16):
                emit_lh(lh, "act" if lh % 2 == 0 else "vec")
                if lh == 11:
                    emit_y(0, 512, 768)
                    emit_y(1, 512, 768)
                elif lh == 13:
                    emit_y(0, 768, 896)
                    emit_y(1, 768, 896)
            emit_y(0, 896, 1024)
            emit_y(1, 896, 1024)

    nc.compile()
    return nc


def _get_nc():
    if "nc" not in _cached:
        _cached["nc"] = _build_nc()
    return _cached["nc"]


LAST_EXEC_NS = None
LAST_TRACE_DIR = None


def kernel(inp, w_theta, b_theta, w_phi, b_phi, w_g, b_g, w_out, b_out):
    import os
    from concourse.bass_utils import run_bass_kernel_spmd

    nc = _get_nc()

    f = np.float32
    c = np.ascontiguousarray

    # [w_theta | w_phi] concatenated, as (c_lo, half, 256) fp16
    wcat = np.concatenate([w_theta.T, w_phi.T * 0.5], axis=1).astype(f)
    wc3 = c(wcat.reshape(2, 128, 256).transpose(1, 0, 2).astype(np.float16))
    wg3 = c(w_g.T.reshape(2, 128, CH).transpose(1, 0, 2).astype(np.float16))
    wo3 = c(w_out.reshape(2, 128, CH).transpose(2, 0, 1).astype(f))  # [ch, m, co]
    bias2v = c(np.tile(np.concatenate([b_theta, b_phi * 0.5]).astype(f),
                       (64, 1)))
    bg1 = c(b_g.astype(f)[:, None])
    bo2 = c(b_out.reshape(2, 128).T.astype(f))
    ident = np.eye(128, dtype=f)

    in_maps = []
    for core in range(8):
        b, k = core // 4, core % 4
        x = inp[b].reshape(C, H, W).astype(f)
        # own h-block first, then the rest: kernel is h-order agnostic
        perm = list(range(HS * k, HS * (k + 1))) + \
            [h for h in range(H) if not (HS * k <= h < HS * (k + 1))]
        xp = x[:, perm, :].reshape(C, H * W)
        xp16 = xp.astype(np.float16)
        in_maps.append({
            "x0": c(xp16[:128]), "x1": c(xp16[128:]),
            "xs0": c(xp[:128, :1024]), "xs1": c(xp[128:, :1024]),
            "wc": wc3, "wg": wg3, "wo": wo3,
            "bias2": bias2v, "bg": bg1, "bo": bo2, "ident": ident,
        })

    trace = bool(os.environ.get("NLB_TRACE"))
    tmpdir = os.environ.get("NLB_TRACE_DIR") or None
    res = run_bass_kernel_spmd(nc, in_maps, list(range(8)), trace=trace,
                               tmpdir=tmpdir)
    global LAST_EXEC_NS, LAST_TRACE_DIR
    LAST_EXEC_NS = res.exec_time_ns
    LAST_TRACE_DIR = tmpdir

    out = np.empty((B, C, H, W), dtype=f)
    for core in range(8):
        b, k = core // 4, core % 4
        yc = res.results[core]["y"].reshape(C, HS, W)
        out[b, :, HS * k:HS * (k + 1), :] = yc
    return out
